# revision 32
# baseline (speedup 1.0000x reference)
"""Cross-covariance (XCA / channel) attention kernel for Trainium2, 8 NeuronCores.

Reference computation (per batch b, head h, with X = x[b] in R^{N x C}):
    qkv = X @ Wqkv + bqkv;  q,k,v per head as [hd, N] (channels x tokens)
    q <- l2norm(q, axis=N) * temp_h ; k <- l2norm(k, axis=N)
    attn = softmax(q @ k^T)                # [hd, hd] channel attention
    out_h = attn @ v                       # [hd, N]
    y = concat_h(out_h)^T @ Wproj + bproj  # [N, C]

Restructure (mathematically exact): all attention statistics derive from the
per-batch Gram matrix S = X^T X in R^{C x C}:
    G[h] = Wq_h^T S Wk_h,  ||q_d||^2 = diag(Wq_h^T S Wq_h),  likewise k
    attn[h] = softmax(temp_h * G[h] / (||q|| ||k||^T))
    y = X @ M, where M = sum_h Wv_h @ attn[h]^T @ Wproj_h

Sharding (v3): 8 cores = 4 batches x 2 token-halves, FULLY INDEPENDENT —
no collectives. v2 split S by column halves and pair-AllGathered partials;
NTFF profiling showed the real cost is not the 200 KB exchange (~6 us) but
the NEFF-start barrier (19-155 us, absorbing PJRT launch skew) plus skewed
collective waits (up to 123 us). v3 instead computes the FULL per-batch
statistics on each pair member and removes every cross-core dependency.

The duplicated Gram work is paid for with a triangular trick: S is
symmetric, so each core accumulates only the upper-triangular block rows
S[chunk_i, 128*i:768] — 2688 of 4608 column-stripes — which needs exactly
the 8 PSUM banks. Lower blocks are PE-transposes of upper ones.

Precision: x streams as bf16; S/U/attn/weights/M all bf16 (full PE rate;
PSUM accumulates fp32); y stores bf16 and is upcast on host. Measured
rel-err ~5e-3 vs the 2e-2 gate.

DMA: packed DRAM layouts give >=3KB contiguous per-partition runs (packet
rate, not bytes, limits DGE queues). x stream alternates sync/vector
queues; weights then x^T stream on the scalar queue; y stores on sync.
"""
import numpy as np
import ml_dtypes

import concourse.bacc as bacc
import concourse.mybir as mybir
import concourse.tile as tile

B, N, C = 4, 8192, 768
H, HD = 12, 64
NLOC = N // 2          # tokens per core (4096)
NCORES = 8
KC = C // 128          # 6 channel chunks
FH = C // 2            # 384
NG = 16                # x-stream groups (4 x 128 tokens each)
XTT = 8                # xT-stream tiles (512 tokens each)
F32 = mybir.dt.float32
F32R = mybir.dt.float32r
BF16 = mybir.dt.bfloat16
AX = mybir.AxisListType.X
BFNP = ml_dtypes.bfloat16

# phase-1 PSUM bank plan: (row-chunk i, col_start, col_end) per bank.
# Upper-triangular stripes S[128i:128(i+1), c0:c1]; 2688 columns total.
SEGS = [(0, 0, 512), (0, 512, 768),
        (1, 128, 640), (1, 640, 768),
        (2, 256, 768), (3, 384, 768),
        (4, 512, 768), (5, 640, 768)]

_CACHE = {}


def _build():
    nc = bacc.Bacc("TRN2", target_bir_lowering=False, debug=False,
                   enable_asserts=False, num_devices=NCORES)

    # ---- per-core I/O (packed layouts; >=3KB contiguous per partition) ----
    xq_d = nc.dram_tensor("xq", [128, NG, 4, C], BF16, kind="ExternalInput")
    xtq_d = nc.dram_tensor("xtq", [128, XTT, KC, 512], BF16, kind="ExternalInput")
    wq_d = nc.dram_tensor("wq", [128, KC, C], BF16, kind="ExternalInput")
    wk_d = nc.dram_tensor("wk", [128, KC, C], BF16, kind="ExternalInput")
    wvth_d = nc.dram_tensor("wvth", [HD, H, C], BF16, kind="ExternalInput")
    wprojh_d = nc.dram_tensor("wprojh", [HD, H, C], BF16, kind="ExternalInput")
    tempdh_d = nc.dram_tensor("tempdh", [HD, H], F32, kind="ExternalInput")
    ones128_d = nc.dram_tensor("ones128", [128, 1], BF16, kind="ExternalInput")
    ones1_d = nc.dram_tensor("ones1", [1, HD], BF16, kind="ExternalInput")
    ident_d = nc.dram_tensor("ident", [128, 128], BF16, kind="ExternalInput")
    # y packed as [p, s, j, c] = y_local[(2s+j)*128 + p, c]: 3KB/partition runs
    y_d = nc.dram_tensor("y", [128, 2 * XTT, 2, C], BF16, kind="ExternalOutput")

    with tile.TileContext(nc) as tc:
        with (
            tc.tile_pool(name="w", bufs=1) as wpool,       # weights
            tc.tile_pool(name="xs", bufs=6) as xs,         # x stream
            tc.tile_pool(name="xts", bufs=3) as xts,       # xT stream
            tc.tile_pool(name="sb", bufs=1) as sb,         # S / U / M
            tc.tile_pool(name="yo", bufs=3) as yo,         # y out
            tc.tile_pool(name="small", bufs=1) as small,
            tc.tile_pool(name="const", bufs=1) as const,
            tc.tile_pool(name="ps", bufs=8, space="PSUM") as ps,
            tc.tile_pool(name="dram", bufs=1, space="DRAM") as dram,
        ):
            # ---- const loads (gpsimd queue, before the x thirds) ----
            ident_sb = const.tile([128, 128], BF16, tag="ident")
            nc.gpsimd.dma_start(out=ident_sb[:, :], in_=ident_d[:, :])
            ones128_sb = const.tile([128, 1], BF16, tag="ones128")
            nc.gpsimd.dma_start(out=ones128_sb[:, :], in_=ones128_d[:, :])
            ones1_sb = const.tile([1, HD], BF16, tag="ones1")
            nc.gpsimd.dma_start(out=ones1_sb[:, :], in_=ones1_d[:, :])
            tempdh_sb = const.tile([HD, H], F32, tag="tempdh")
            nc.gpsimd.dma_start(out=tempdh_sb[:, :], in_=tempdh_d[:, :])

            # ---- phase 1: upper-triangular S stripes over all 8192 tokens ----
            # x stream rotates across all three DGE queues. Full 768KB
            # transfers only: smaller first transfers were tried and clogged
            # the queue (1536B packets move at ~40GB/s vs ~150GB/s at 6KB).
            s_ps = [ps.tile([128, c1 - c0], F32, tag="ps", name=f"s_ps{si}")
                    for si, (i, c0, c1) in enumerate(SEGS)]
            qplan = "SSAS AGSA GSAG SAGA".replace(" ", "")
            for g in range(NG):
                x_t = xs.tile([128, 4, C], BF16, tag="xs")
                eng = {"S": nc.sync, "A": nc.scalar, "G": nc.gpsimd}[qplan[g]]
                eng.dma_start(out=x_t[:, :, :], in_=xq_d[:, g, :, :])
                for j in range(4):
                    first = (g == 0 and j == 0)
                    last = (g == NG - 1 and j == 3)
                    for si, (i, c0, c1) in enumerate(SEGS):
                        nc.tensor.matmul(s_ps[si][:, :],
                                         x_t[:, j, 128 * i:128 * (i + 1)],
                                         x_t[:, j, c0:c1],
                                         start=first, stop=last)

            # weights trail the x thirds on the gpsimd queue (needed at U-time)
            wk_sb = wpool.tile([128, KC, C], BF16, tag="wk")
            nc.gpsimd.dma_start(out=wk_sb[:, :, :], in_=wk_d[:, :, :])
            wq_sb = wpool.tile([128, KC, C], BF16, tag="wq")
            nc.gpsimd.dma_start(out=wq_sb[:, :, :], in_=wq_d[:, :, :])
            wprojh_sb = wpool.tile([HD, H, C], BF16, tag="wprojh")
            nc.gpsimd.dma_start(out=wprojh_sb[:, :, :], in_=wprojh_d[:, :, :])
            wvth_sb = wpool.tile([HD, H, C], BF16, tag="wvth")
            nc.gpsimd.dma_start(out=wvth_sb[:, :, :], in_=wvth_d[:, :, :])

            # ---- reconstruct full S (bf16) in SBUF; lower = transpose(upper) ----
            s_sb = sb.tile([128, KC, C], BF16, tag="s")
            for si, (i, c0, c1) in enumerate(SEGS):
                nc.vector.tensor_copy(s_sb[:, i, c0:c1], s_ps[si][:, :])
            for i in range(1, KC):
                for jj in range(i):
                    t_ps = ps.tile([128, 128], BF16, tag="ps")
                    nc.tensor.transpose(t_ps[:, :],
                                        s_sb[:, jj, 128 * i:128 * (i + 1)],
                                        ident_sb[:, :])
                    nc.vector.tensor_copy(s_sb[:, i, 128 * jj:128 * (jj + 1)],
                                          t_ps[:, :])

            # ---- U[di] = S @ W[di] (di: 0=q, 1=k), norms, G, softmax scales ----
            # PE order interleaves the norm matmuls into the U/G stream so the
            # scalar/DVE chains (rsqrt, [1,C]->[d,h] DRAM round-trip, scale
            # precompute) hide behind matmul work and the PE never idles long.
            u_sb = sb.tile([128, KC, 2, C], BF16, tag="u")
            pr_sb = [None, None]
            n_ps = {}

            def u_half(di, w_sb, f):
                for m in range(KC):
                    u_ps = ps.tile([128, FH], F32, tag="ps")
                    for k in range(KC):
                        nc.tensor.matmul(u_ps[:, :],
                                         s_sb[:, k, 128 * m:128 * (m + 1)],
                                         w_sb[:, k, f * FH:(f + 1) * FH],
                                         start=(k == 0), stop=(k == KC - 1))
                    nc.vector.tensor_copy(u_sb[:, m, di, f * FH:(f + 1) * FH],
                                          u_ps[:, :])

            def pr_mul(di, w_sb, f):
                if pr_sb[di] is None:
                    pr_sb[di] = sb.tile([128, KC, C], BF16, tag=f"pr{di}",
                                        name=f"pr{di}")
                nc.vector.tensor_mul(pr_sb[di][:, :, f * FH:(f + 1) * FH],
                                     w_sb[:, :, f * FH:(f + 1) * FH],
                                     u_sb[:, :, di, f * FH:(f + 1) * FH])

            def norm_mms(di, f):
                p = ps.tile([1, FH], F32, tag="ps", name=f"n_ps{di}{f}")
                for k in range(KC):
                    nc.tensor.matmul(p[:, :], ones128_sb[:, :],
                                     pr_sb[di][:, k, f * FH:(f + 1) * FH],
                                     start=(k == 0), stop=(k == KC - 1))
                n_ps[(di, f)] = p

            u_half(1, wk_sb, 0)
            u_half(1, wk_sb, 1)
            pr_mul(1, wk_sb, 0)
            pr_mul(1, wk_sb, 1)
            u_half(0, wq_sb, 0)
            norm_mms(1, 0)           # nk2 halves (pr1 ready during U(0))
            norm_mms(1, 1)
            pr_mul(0, wq_sb, 0)
            # rinv_k = 1/sqrt(nk2) straight off PSUM (scalar table op), then
            # broadcast to 64 partitions via K=1 matmuls.
            rinvk_sb = small.tile([1, C], BF16, tag="rinvk")
            for f in range(2):
                nc.scalar.activation(rinvk_sb[:, f * FH:(f + 1) * FH],
                                     n_ps[(1, f)][:, :],
                                     mybir.ActivationFunctionType.Abs_reciprocal_sqrt)
            # nq2 [1,(h d)] -> [d, h] via PE transposes of [1,128] slices
            # (on-chip; the old DRAM round-trip cost ~6us of DMA latency
            # on the pre-softmax critical chain)
            nq2_sb = small.tile([1, C], BF16, tag="nq2")
            nq2T_sb = small.tile([HD, H], BF16, tag="nq2T")
            tnqT_sb = small.tile([HD, H], F32, tag="tnqT")

            def nq_chain(f):
                nc.vector.tensor_copy(nq2_sb[:, f * FH:(f + 1) * FH],
                                      n_ps[(0, f)][:, :])
                for i in range(3):
                    c0 = f * FH + 128 * i
                    tp = ps.tile([128, 1], BF16, tag="ps")
                    nc.tensor.transpose(tp[:, :], nq2_sb[:, c0:c0 + 128],
                                        ident_sb[0:1, 0:1])
                    h = 6 * f + 2 * i
                    nc.vector.tensor_copy(nq2T_sb[:, h:h + 1], tp[0:HD, :])
                    nc.vector.tensor_copy(nq2T_sb[:, h + 1:h + 2], tp[HD:128, :])

            u_half(0, wq_sb, 1)
            norm_mms(0, 0)
            nq_chain(0)
            pr_mul(0, wq_sb, 1)
            norm_mms(0, 1)
            nq_chain(1)
            for f in range(2):
                nc.scalar.activation(tnqT_sb[:, 6 * f:6 * (f + 1)],
                                     nq2T_sb[:, 6 * f:6 * (f + 1)],
                                     mybir.ActivationFunctionType.Abs_reciprocal_sqrt)
            # preload the Exp table now that all rsqrt activations are done
            # (loads during the G matmuls, not inside the softmax chain)
            dump2_sb = small.tile([HD, H], F32, tag="dump2")
            nc.scalar.activation(dump2_sb[:, :], tempdh_sb[:, :],
                                 mybir.ActivationFunctionType.Exp)
            # nkbc copies precede every tnqT-dependent DVE op so they are not
            # queued behind the (later-arriving) tnqT chain
            nkbc_sb = small.tile([HD, C], F32, tag="nkbc")
            for f in range(2):
                b_ps = ps.tile([HD, FH], F32, tag="ps", name=f"nkbc{f}")
                nc.tensor.matmul(b_ps[:, :], ones1_sb[:, :],
                                 rinvk_sb[:, f * FH:(f + 1) * FH],
                                 start=True, stop=True)
                nc.vector.tensor_copy(nkbc_sb[:, f * FH:(f + 1) * FH], b_ps[:, :])
            nc.vector.tensor_mul(tnqT_sb[:, :], tnqT_sb[:, :], tempdh_sb[:, :])
            # scale[d, h, e] = temp_h/nq[d,h] * 1/nk[e,h] — precomputed so the
            # per-half softmax chain is a single PSUM multiply + exp + norm
            scale_sb = small.tile([HD, H, HD], F32, tag="scale")
            nc.vector.tensor_mul(
                scale_sb[:, :, :],
                nkbc_sb.rearrange("d (h e) -> d h e", h=H),
                tnqT_sb.unsqueeze(2).broadcast_to([HD, H, HD]))

            # ---- G[h] = Wq_h^T Uk_h, softmax fully hidden under G/R matmuls ----
            # softmax(hf) is emitted right after G(hf)'s PSUM copy, so half 0's
            # chain runs on DVE/scalar while the PE grinds G half 1 — R then
            # starts with no PE idle (an idle >3us here re-throttles the PE
            # clock for ~60us; see the HAM section of the tensor-engine guide).
            # |logits| <= max|temp| (Cauchy-Schwarz on normalized vectors):
            # safe to exp without max-subtraction for the given inputs.
            t1_sb = small.tile([HD, H, HD], F32, tag="t1")
            e_sb = small.tile([HD, H, HD], F32, tag="e")
            sum_sb = small.tile([HD, H], F32, tag="sum")
            rec_sb = small.tile([HD, H], F32, tag="rec")
            attn_sb = small.tile([HD, H, HD], BF16, tag="attn")
            r_sb = sb.tile([HD, H, C], BF16, tag="r")
            for hf in range(2):
                h0 = hf * 6
                g_ps = ps.tile([HD, FH], F32, tag="ps")
                for hh in range(6):
                    h = h0 + hh
                    for k in range(KC):
                        nc.tensor.matmul(g_ps[:, hh * HD:(hh + 1) * HD],
                                         wq_sb[:, k, h * HD:(h + 1) * HD],
                                         u_sb[:, k, 1, h * HD:(h + 1) * HD],
                                         start=(k == 0), stop=(k == KC - 1))
                # logits = G * scale straight off PSUM (no g copy needed)
                nc.vector.tensor_mul(
                    t1_sb[:, h0:h0 + 6, :],
                    g_ps.rearrange("d (h e) -> d h e", h=6),
                    scale_sb[:, h0:h0 + 6, :])
                nc.scalar.activation(e_sb[:, h0:h0 + 6, :], t1_sb[:, h0:h0 + 6, :],
                                     mybir.ActivationFunctionType.Exp)
                nc.vector.reduce_sum(sum_sb[:, h0:h0 + 6], e_sb[:, h0:h0 + 6, :], AX)
                nc.vector.reciprocal(rec_sb[:, h0:h0 + 6], sum_sb[:, h0:h0 + 6])
                nc.vector.tensor_mul(
                    attn_sb[:, h0:h0 + 6, :], e_sb[:, h0:h0 + 6, :],
                    rec_sb[:, h0:h0 + 6].unsqueeze(2).broadcast_to([HD, 6, HD]))

            # ---- R_h = attn_h^T @ Wproj_h (copies split DVE/scalar so the
            # M accumulation is not paced by a single copy engine) ----
            for h in range(H):
                for f in range(2):
                    r_ps = ps.tile([HD, FH], F32, tag="ps")
                    nc.tensor.matmul(r_ps[:, :], attn_sb[:, h, :],
                                     wprojh_sb[:, h, f * FH:(f + 1) * FH],
                                     start=True, stop=True)
                    if (h + f) % 2 == 0:
                        nc.vector.tensor_copy(r_sb[:, h, f * FH:(f + 1) * FH],
                                              r_ps[:, :])
                    else:
                        nc.scalar.activation(r_sb[:, h, f * FH:(f + 1) * FH],
                                             r_ps[:, :],
                                             mybir.ActivationFunctionType.Copy)

            # ---- M = sum_h Wv_h @ R_h   [C, C] bf16 ----
            m_sb = sb.tile([128, KC, C], BF16, tag="m")
            for m in range(KC):
                for f in range(2):
                    m_ps = ps.tile([128, FH], F32, tag="ps")
                    for h in range(H):
                        nc.tensor.matmul(m_ps[:, :],
                                         wvth_sb[:, h, 128 * m:128 * (m + 1)],
                                         r_sb[:, h, f * FH:(f + 1) * FH],
                                         start=(h == 0), stop=(h == H - 1))
                    if (m + f) % 2 == 0:
                        nc.vector.tensor_copy(m_sb[:, m, f * FH:(f + 1) * FH],
                                              m_ps[:, :])
                    else:
                        nc.scalar.activation(m_sb[:, m, f * FH:(f + 1) * FH],
                                             m_ps[:, :],
                                             mybir.ActivationFunctionType.Copy)

            # ---- phase 4: y = x_half @ M (y stored packed, 2 tiles/store) ----
            for t in range(XTT):
                xt_t = xts.tile([128, KC, 512], BF16, tag="xt")
                nc.gpsimd.dma_start(out=xt_t[:, :, :], in_=xtq_d[:, t, :, :])
                for sp in range(2):
                    y_sb = yo.tile([128, 2, C], BF16, tag="y")
                    for j in range(2):
                        sub = sp * 2 + j
                        for f in range(2):
                            y_ps = ps.tile([128, FH], F32, tag="ps")
                            for k in range(KC):
                                nc.tensor.matmul(
                                    y_ps[:, :],
                                    xt_t[:, k, 128 * sub:128 * (sub + 1)],
                                    m_sb[:, k, f * FH:(f + 1) * FH],
                                    start=(k == 0), stop=(k == KC - 1))
                            if f == 0:
                                nc.vector.tensor_copy(
                                    y_sb[:, j, f * FH:(f + 1) * FH], y_ps[:, :])
                            else:
                                nc.scalar.activation(
                                    y_sb[:, j, f * FH:(f + 1) * FH], y_ps[:, :],
                                    mybir.ActivationFunctionType.Copy)
                    yeng = nc.sync if sp == 0 else nc.scalar
                    yeng.dma_start(out=y_d[:, t * 2 + sp, :, :],
                                   in_=y_sb[:, :, :])

    nc.compile()
    return nc


def _get_program(has_bias: bool = False):
    if "v3" not in _CACHE:
        _CACHE["v3"] = _build()
    return _CACHE["v3"]


def _prepare_inputs(x, Wqkv, bqkv, temperature, Wproj, bproj, has_bias=False):
    """Build the 8 per-core input maps (host-side packing + dtype prep)."""
    x = np.asarray(x, np.float32)
    Wqkv = np.asarray(Wqkv, np.float32)
    temperature = np.asarray(temperature, np.float32)
    Wproj = np.asarray(Wproj, np.float32)

    Wq = Wqkv[:, :C]
    Wk = Wqkv[:, C:2 * C]
    Wv = Wqkv[:, 2 * C:]
    # [128, KC, C]: wqp[p, k, c] = Wq[k*128+p, c]
    wqp = np.ascontiguousarray(
        Wq.reshape(KC, 128, C).transpose(1, 0, 2)).astype(BFNP)
    wkp = np.ascontiguousarray(
        Wk.reshape(KC, 128, C).transpose(1, 0, 2)).astype(BFNP)
    # wvth[e, h, c] = Wv[c, 64h+e];  wprojh[d, h, c] = Wproj[64h+d, c]
    wvth = np.ascontiguousarray(
        Wv.reshape(C, H, HD).transpose(2, 1, 0)).astype(BFNP)
    wprojh = np.ascontiguousarray(
        Wproj.reshape(H, HD, C).transpose(1, 0, 2)).astype(BFNP)
    tempdh = np.tile(temperature.reshape(1, H), (HD, 1)).astype(np.float32)

    common = dict(wq=wqp, wk=wkp, wvth=wvth, wprojh=wprojh, tempdh=tempdh,
                  ones128=np.ones((128, 1), BFNP),
                  ones1=np.ones((1, HD), BFNP),
                  ident=np.eye(128, dtype=np.float32).astype(BFNP))

    # xq[p, g, j, c] = x[b, g*512 + j*128 + p, c] — shared by the batch pair
    xqs = [np.ascontiguousarray(
        x[b].reshape(NG, 4, 128, C).transpose(2, 0, 1, 3)).astype(BFNP)
        for b in range(B)]

    in_maps = []
    for core in range(NCORES):
        b, j = core // 2, core % 2
        xh = x[b, j * NLOC:(j + 1) * NLOC]
        # xtq[p, t, k, n] = xh[t*512 + n, k*128 + p]
        xtq = np.ascontiguousarray(
            xh.reshape(XTT, 512, KC, 128).transpose(3, 0, 2, 1)).astype(BFNP)
        m = dict(common)
        m["xq"] = xqs[b]
        m["xtq"] = xtq
        in_maps.append(m)
    return in_maps


def _reference_host(x, Wqkv, bqkv, temperature, Wproj, bproj):
    """Exact numpy fallback (used only for nonzero biases)."""
    x = np.asarray(x, np.float64)
    Wqkv = np.asarray(Wqkv, np.float64)
    bqkv = np.asarray(bqkv, np.float64)
    temperature = np.asarray(temperature, np.float64)
    Wproj = np.asarray(Wproj, np.float64)
    bproj = np.asarray(bproj, np.float64)
    Bq, Nq, Cq = x.shape
    hd = Cq // H
    qkv = (x @ Wqkv + bqkv).reshape(Bq, Nq, 3, H, hd)
    qkv = qkv.transpose(2, 0, 3, 4, 1)
    q, k, v = qkv[0], qkv[1], qkv[2]

    def l2n(t):
        n = np.sqrt((t * t).sum(axis=-1, keepdims=True))
        return t / np.maximum(n, 1e-12)

    q = l2n(q) * temperature
    k = l2n(k)
    logits = np.einsum('bhdn,bhen->bhde', q, k)
    logits -= logits.max(axis=-1, keepdims=True)
    e = np.exp(logits)
    attn = e / e.sum(axis=-1, keepdims=True)
    out = np.einsum('bhde,bhen->bhdn', attn, v)
    out = out.transpose(0, 3, 1, 2).reshape(Bq, Nq, Cq)
    return (out @ Wproj + bproj).astype(np.float32)


def kernel(x, Wqkv, bqkv, temperature, Wproj, bproj):
    has_bias = bool(np.any(np.asarray(bqkv)) or np.any(np.asarray(bproj)))
    if has_bias:
        return _reference_host(x, Wqkv, bqkv, temperature, Wproj, bproj)
    from concourse import bass2jax
    nc = _get_program(False)
    in_maps = _prepare_inputs(x, Wqkv, bqkv, temperature, Wproj, bproj, False)
    results = bass2jax.run_bass_via_pjrt(nc, in_maps, n_cores=NCORES)
    out = np.empty((B, N, C), np.float32)
    for core in range(NCORES):
        b, j = core // 2, core % 2
        # y packed [128, 16, 2, C]: row = s*256 + jj*128 + p
        yp = results[core]["y"].astype(np.float32)
        out[b, j * NLOC:(j + 1) * NLOC, :] = (
            yp.transpose(1, 2, 0, 3).reshape(NLOC, C))
    return out


# revision 34
# speedup vs baseline: 1.0274x; 1.0274x over previous
"""Cross-covariance (XCA / channel) attention kernel for Trainium2, 8 NeuronCores.

Reference computation (per batch b, head h, with X = x[b] in R^{N x C}):
    qkv = X @ Wqkv + bqkv;  q,k,v per head as [hd, N] (channels x tokens)
    q <- l2norm(q, axis=N) * temp_h ; k <- l2norm(k, axis=N)
    attn = softmax(q @ k^T)                # [hd, hd] channel attention
    out_h = attn @ v                       # [hd, N]
    y = concat_h(out_h)^T @ Wproj + bproj  # [N, C]

Restructure (mathematically exact): all attention statistics derive from the
per-batch Gram matrix S = X^T X in R^{C x C}:
    G[h] = Wq_h^T S Wk_h,  ||q_d||^2 = diag(Wq_h^T S Wq_h),  likewise k
    attn[h] = softmax(temp_h * G[h] / (||q|| ||k||^T))
    y = X @ M, where M = sum_h Wv_h @ attn[h]^T @ Wproj_h

Sharding (v3): 8 cores = 4 batches x 2 token-halves, FULLY INDEPENDENT —
no collectives. v2 split S by column halves and pair-AllGathered partials;
NTFF profiling showed the real cost is not the 200 KB exchange (~6 us) but
the NEFF-start barrier (19-155 us, absorbing PJRT launch skew) plus skewed
collective waits (up to 123 us). v3 instead computes the FULL per-batch
statistics on each pair member and removes every cross-core dependency.

The duplicated Gram work is paid for with a triangular trick: S is
symmetric, so each core accumulates only the upper-triangular block rows
S[chunk_i, 128*i:768] — 2688 of 4608 column-stripes — which needs exactly
the 8 PSUM banks. Lower blocks are PE-transposes of upper ones.

Precision: x streams as bf16; S/U/attn/weights/M all bf16 (full PE rate;
PSUM accumulates fp32); y stores bf16 and is upcast on host. Measured
rel-err ~5e-3 vs the 2e-2 gate.

DMA: packed DRAM layouts give >=3KB contiguous per-partition runs (packet
rate, not bytes, limits DGE queues). x stream alternates sync/vector
queues; weights then x^T stream on the scalar queue; y stores on sync.
"""
import numpy as np
import ml_dtypes

import concourse.bacc as bacc
import concourse.mybir as mybir
import concourse.tile as tile

B, N, C = 4, 8192, 768
H, HD = 12, 64
NLOC = N // 2          # tokens per core (4096)
NCORES = 8
KC = C // 128          # 6 channel chunks
FH = C // 2            # 384
NG = 16                # x-stream groups (4 x 128 tokens each)
XTT = 8                # xT-stream tiles (512 tokens each)
F32 = mybir.dt.float32
F32R = mybir.dt.float32r
BF16 = mybir.dt.bfloat16
AX = mybir.AxisListType.X
BFNP = ml_dtypes.bfloat16

# phase-1 PSUM bank plan: (row-chunk i, col_start, col_end) per bank.
# Upper-triangular stripes S[128i:128(i+1), c0:c1]; 2688 columns total.
SEGS = [(0, 0, 512), (0, 512, 768),
        (1, 128, 640), (1, 640, 768),
        (2, 256, 768), (3, 384, 768),
        (4, 512, 768), (5, 640, 768)]

_CACHE = {}


def _build():
    nc = bacc.Bacc("TRN2", target_bir_lowering=False, debug=False,
                   enable_asserts=False, num_devices=NCORES)

    # ---- per-core I/O (packed layouts; >=3KB contiguous per partition) ----
    xq_d = nc.dram_tensor("xq", [128, NG, 4, C], BF16, kind="ExternalInput")
    xtq_d = nc.dram_tensor("xtq", [128, XTT, KC, 512], BF16, kind="ExternalInput")
    wq_d = nc.dram_tensor("wq", [128, KC, C], BF16, kind="ExternalInput")
    wk_d = nc.dram_tensor("wk", [128, KC, C], BF16, kind="ExternalInput")
    wvth_d = nc.dram_tensor("wvth", [HD, H, C], BF16, kind="ExternalInput")
    wprojh_d = nc.dram_tensor("wprojh", [HD, H, C], BF16, kind="ExternalInput")
    tempdh_d = nc.dram_tensor("tempdh", [HD, H], F32, kind="ExternalInput")
    ones128_d = nc.dram_tensor("ones128", [128, 1], BF16, kind="ExternalInput")
    ones1_d = nc.dram_tensor("ones1", [1, HD], BF16, kind="ExternalInput")
    ident_d = nc.dram_tensor("ident", [128, 128], BF16, kind="ExternalInput")
    # y packed as [p, s, j, c] = y_local[(2s+j)*128 + p, c]: 3KB/partition runs
    y_d = nc.dram_tensor("y", [128, 2 * XTT, 2, C], BF16, kind="ExternalOutput")

    with tile.TileContext(nc) as tc:
        with (
            tc.tile_pool(name="w", bufs=1) as wpool,       # weights
            tc.tile_pool(name="xs", bufs=6) as xs,         # x stream
            tc.tile_pool(name="xts", bufs=3) as xts,       # xT stream
            tc.tile_pool(name="sb", bufs=1) as sb,         # S / U / M
            tc.tile_pool(name="yo", bufs=3) as yo,         # y out
            tc.tile_pool(name="small", bufs=1) as small,
            tc.tile_pool(name="const", bufs=1) as const,
            tc.tile_pool(name="ps", bufs=8, space="PSUM") as ps,
            tc.tile_pool(name="dram", bufs=1, space="DRAM") as dram,
        ):
            # ---- const loads (gpsimd queue, before the x thirds) ----
            ident_sb = const.tile([128, 128], BF16, tag="ident")
            nc.gpsimd.dma_start(out=ident_sb[:, :], in_=ident_d[:, :])
            ones128_sb = const.tile([128, 1], BF16, tag="ones128")
            nc.gpsimd.dma_start(out=ones128_sb[:, :], in_=ones128_d[:, :])
            ones1_sb = const.tile([1, HD], BF16, tag="ones1")
            nc.gpsimd.dma_start(out=ones1_sb[:, :], in_=ones1_d[:, :])
            tempdh_sb = const.tile([HD, H], F32, tag="tempdh")
            nc.gpsimd.dma_start(out=tempdh_sb[:, :], in_=tempdh_d[:, :])

            # ---- phase 1: upper-triangular S stripes over all 8192 tokens ----
            # x stream rotates across all three DGE queues. Full 768KB
            # transfers only: smaller first transfers were tried and clogged
            # the queue (1536B packets move at ~40GB/s vs ~150GB/s at 6KB).
            s_ps = [ps.tile([128, c1 - c0], F32, tag="ps", name=f"s_ps{si}")
                    for si, (i, c0, c1) in enumerate(SEGS)]
            qplan = "SAGS AGSA GSAG SAGS".replace(" ", "")
            for g in range(NG):
                x_t = xs.tile([128, 4, C], BF16, tag="xs")
                eng = {"S": nc.sync, "A": nc.scalar, "G": nc.gpsimd}[qplan[g]]
                eng.dma_start(out=x_t[:, :, :], in_=xq_d[:, g, :, :])
                for j in range(4):
                    first = (g == 0 and j == 0)
                    last = (g == NG - 1 and j == 3)
                    for si, (i, c0, c1) in enumerate(SEGS):
                        nc.tensor.matmul(s_ps[si][:, :],
                                         x_t[:, j, 128 * i:128 * (i + 1)],
                                         x_t[:, j, c0:c1],
                                         start=first, stop=last)

            # weights trail the x thirds on the gpsimd queue (needed at U-time)
            wk_sb = wpool.tile([128, KC, C], BF16, tag="wk")
            nc.gpsimd.dma_start(out=wk_sb[:, :, :], in_=wk_d[:, :, :])
            wq_sb = wpool.tile([128, KC, C], BF16, tag="wq")
            nc.gpsimd.dma_start(out=wq_sb[:, :, :], in_=wq_d[:, :, :])
            wprojh_sb = wpool.tile([HD, H, C], BF16, tag="wprojh")
            nc.gpsimd.dma_start(out=wprojh_sb[:, :, :], in_=wprojh_d[:, :, :])
            wvth_sb = wpool.tile([HD, H, C], BF16, tag="wvth")
            nc.gpsimd.dma_start(out=wvth_sb[:, :, :], in_=wvth_d[:, :, :])

            # ---- reconstruct full S (bf16) in SBUF; lower = transpose(upper) ----
            s_sb = sb.tile([128, KC, C], BF16, tag="s")
            for si, (i, c0, c1) in enumerate(SEGS):
                nc.vector.tensor_copy(s_sb[:, i, c0:c1], s_ps[si][:, :])
            for i in range(1, KC):
                for jj in range(i):
                    t_ps = ps.tile([128, 128], BF16, tag="ps")
                    nc.tensor.transpose(t_ps[:, :],
                                        s_sb[:, jj, 128 * i:128 * (i + 1)],
                                        ident_sb[:, :])
                    nc.vector.tensor_copy(s_sb[:, i, 128 * jj:128 * (jj + 1)],
                                          t_ps[:, :])

            # ---- U[di] = S @ W[di] (di: 0=q, 1=k), norms, G, softmax scales ----
            # PE order interleaves the norm matmuls into the U/G stream so the
            # scalar/DVE chains (rsqrt, [1,C]->[d,h] DRAM round-trip, scale
            # precompute) hide behind matmul work and the PE never idles long.
            u_sb = sb.tile([128, KC, 2, C], BF16, tag="u")
            pr_sb = [None, None]
            n_ps = {}

            def u_half(di, w_sb, f):
                for m in range(KC):
                    u_ps = ps.tile([128, FH], F32, tag="ps")
                    for k in range(KC):
                        nc.tensor.matmul(u_ps[:, :],
                                         s_sb[:, k, 128 * m:128 * (m + 1)],
                                         w_sb[:, k, f * FH:(f + 1) * FH],
                                         start=(k == 0), stop=(k == KC - 1))
                    nc.vector.tensor_copy(u_sb[:, m, di, f * FH:(f + 1) * FH],
                                          u_ps[:, :])

            def pr_mul(di, w_sb, f):
                if pr_sb[di] is None:
                    pr_sb[di] = sb.tile([128, KC, C], BF16, tag=f"pr{di}",
                                        name=f"pr{di}")
                nc.vector.tensor_mul(pr_sb[di][:, :, f * FH:(f + 1) * FH],
                                     w_sb[:, :, f * FH:(f + 1) * FH],
                                     u_sb[:, :, di, f * FH:(f + 1) * FH])

            def norm_mms(di, f):
                p = ps.tile([1, FH], F32, tag="ps", name=f"n_ps{di}{f}")
                for k in range(KC):
                    nc.tensor.matmul(p[:, :], ones128_sb[:, :],
                                     pr_sb[di][:, k, f * FH:(f + 1) * FH],
                                     start=(k == 0), stop=(k == KC - 1))
                n_ps[(di, f)] = p

            u_half(1, wk_sb, 0)
            u_half(1, wk_sb, 1)
            pr_mul(1, wk_sb, 0)
            pr_mul(1, wk_sb, 1)
            u_half(0, wq_sb, 0)
            norm_mms(1, 0)           # nk2 halves (pr1 ready during U(0))
            norm_mms(1, 1)
            pr_mul(0, wq_sb, 0)
            # rinv_k = 1/sqrt(nk2) straight off PSUM (scalar table op), then
            # broadcast to 64 partitions via K=1 matmuls.
            rinvk_sb = small.tile([1, C], BF16, tag="rinvk")
            for f in range(2):
                nc.scalar.activation(rinvk_sb[:, f * FH:(f + 1) * FH],
                                     n_ps[(1, f)][:, :],
                                     mybir.ActivationFunctionType.Abs_reciprocal_sqrt)
            # nq2 [1,(h d)] -> [d, h] via PE transposes of [1,128] slices
            # (on-chip; the old DRAM round-trip cost ~6us of DMA latency
            # on the pre-softmax critical chain)
            nq2_sb = small.tile([1, C], BF16, tag="nq2")
            nq2T_sb = small.tile([HD, H], BF16, tag="nq2T")
            tnqT_sb = small.tile([HD, H], F32, tag="tnqT")

            def nq_chain(f):
                nc.vector.tensor_copy(nq2_sb[:, f * FH:(f + 1) * FH],
                                      n_ps[(0, f)][:, :])
                for i in range(3):
                    c0 = f * FH + 128 * i
                    tp = ps.tile([128, 1], BF16, tag="ps")
                    nc.tensor.transpose(tp[:, :], nq2_sb[:, c0:c0 + 128],
                                        ident_sb[0:1, 0:1])
                    h = 6 * f + 2 * i
                    nc.vector.tensor_copy(nq2T_sb[:, h:h + 1], tp[0:HD, :])
                    nc.vector.tensor_copy(nq2T_sb[:, h + 1:h + 2], tp[HD:128, :])

            u_half(0, wq_sb, 1)
            norm_mms(0, 0)
            nq_chain(0)
            pr_mul(0, wq_sb, 1)
            norm_mms(0, 1)
            nq_chain(1)
            for f in range(2):
                nc.scalar.activation(tnqT_sb[:, 6 * f:6 * (f + 1)],
                                     nq2T_sb[:, 6 * f:6 * (f + 1)],
                                     mybir.ActivationFunctionType.Abs_reciprocal_sqrt)
            # preload the Exp table now that all rsqrt activations are done
            # (loads during the G matmuls, not inside the softmax chain)
            dump2_sb = small.tile([HD, H], F32, tag="dump2")
            nc.scalar.activation(dump2_sb[:, :], tempdh_sb[:, :],
                                 mybir.ActivationFunctionType.Exp)
            # nkbc copies precede every tnqT-dependent DVE op so they are not
            # queued behind the (later-arriving) tnqT chain
            nkbc_sb = small.tile([HD, C], F32, tag="nkbc")
            for f in range(2):
                b_ps = ps.tile([HD, FH], F32, tag="ps", name=f"nkbc{f}")
                nc.tensor.matmul(b_ps[:, :], ones1_sb[:, :],
                                 rinvk_sb[:, f * FH:(f + 1) * FH],
                                 start=True, stop=True)
                nc.vector.tensor_copy(nkbc_sb[:, f * FH:(f + 1) * FH], b_ps[:, :])
            nc.vector.tensor_mul(tnqT_sb[:, :], tnqT_sb[:, :], tempdh_sb[:, :])
            # scale[d, h, e] = temp_h/nq[d,h] * 1/nk[e,h] — precomputed so the
            # per-half softmax chain is a single PSUM multiply + exp + norm
            scale_sb = small.tile([HD, H, HD], F32, tag="scale")
            nc.vector.tensor_mul(
                scale_sb[:, :, :],
                nkbc_sb.rearrange("d (h e) -> d h e", h=H),
                tnqT_sb.unsqueeze(2).broadcast_to([HD, H, HD]))

            # ---- G[h] = Wq_h^T Uk_h, softmax fully hidden under G/R matmuls ----
            # softmax(hf) is emitted right after G(hf)'s PSUM copy, so half 0's
            # chain runs on DVE/scalar while the PE grinds G half 1 — R then
            # starts with no PE idle (an idle >3us here re-throttles the PE
            # clock for ~60us; see the HAM section of the tensor-engine guide).
            # |logits| <= max|temp| (Cauchy-Schwarz on normalized vectors):
            # safe to exp without max-subtraction for the given inputs.
            t1_sb = small.tile([HD, H, HD], F32, tag="t1")
            e_sb = small.tile([HD, H, HD], F32, tag="e")
            sum_sb = small.tile([HD, H], F32, tag="sum")
            rec_sb = small.tile([HD, H], F32, tag="rec")
            attn_sb = small.tile([HD, H, HD], BF16, tag="attn")
            r_sb = sb.tile([HD, H, C], BF16, tag="r")
            for hf in range(2):
                h0 = hf * 6
                g_ps = ps.tile([HD, FH], F32, tag="ps")
                for hh in range(6):
                    h = h0 + hh
                    for k in range(KC):
                        nc.tensor.matmul(g_ps[:, hh * HD:(hh + 1) * HD],
                                         wq_sb[:, k, h * HD:(h + 1) * HD],
                                         u_sb[:, k, 1, h * HD:(h + 1) * HD],
                                         start=(k == 0), stop=(k == KC - 1))
                # logits = G * scale straight off PSUM (no g copy needed)
                nc.vector.tensor_mul(
                    t1_sb[:, h0:h0 + 6, :],
                    g_ps.rearrange("d (h e) -> d h e", h=6),
                    scale_sb[:, h0:h0 + 6, :])
                nc.scalar.activation(e_sb[:, h0:h0 + 6, :], t1_sb[:, h0:h0 + 6, :],
                                     mybir.ActivationFunctionType.Exp)
                nc.vector.reduce_sum(sum_sb[:, h0:h0 + 6], e_sb[:, h0:h0 + 6, :], AX)
                nc.vector.reciprocal(rec_sb[:, h0:h0 + 6], sum_sb[:, h0:h0 + 6])
                nc.vector.tensor_mul(
                    attn_sb[:, h0:h0 + 6, :], e_sb[:, h0:h0 + 6, :],
                    rec_sb[:, h0:h0 + 6].unsqueeze(2).broadcast_to([HD, 6, HD]))

            # PE keep-warm filler: the tail of the softmax chain (exp/normalize
            # on scalar+DVE) leaves the PE ~3us idle, which trips the HAM
            # clock throttle and halves matmul rate for the next ~60us. A
            # burst of dependency-free matmuls on constant data bridges it.
            junk_ps = ps.tile([128, 128], F32, tag="ps")
            for _ in range(28):
                nc.tensor.matmul(junk_ps[:, :], ident_sb[:, :], ident_sb[:, :],
                                 start=True, stop=True, skip_group_check=True)

            # ---- R_h = attn_h^T @ Wproj_h (copies split DVE/scalar so the
            # M accumulation is not paced by a single copy engine) ----
            for h in range(H):
                for f in range(2):
                    r_ps = ps.tile([HD, FH], F32, tag="ps")
                    nc.tensor.matmul(r_ps[:, :], attn_sb[:, h, :],
                                     wprojh_sb[:, h, f * FH:(f + 1) * FH],
                                     start=True, stop=True)
                    if (h + f) % 2 == 0:
                        nc.vector.tensor_copy(r_sb[:, h, f * FH:(f + 1) * FH],
                                              r_ps[:, :])
                    else:
                        nc.scalar.activation(r_sb[:, h, f * FH:(f + 1) * FH],
                                             r_ps[:, :],
                                             mybir.ActivationFunctionType.Copy)

            # ---- M = sum_h Wv_h @ R_h   [C, C] bf16 ----
            m_sb = sb.tile([128, KC, C], BF16, tag="m")
            for m in range(KC):
                for f in range(2):
                    m_ps = ps.tile([128, FH], F32, tag="ps")
                    for h in range(H):
                        nc.tensor.matmul(m_ps[:, :],
                                         wvth_sb[:, h, 128 * m:128 * (m + 1)],
                                         r_sb[:, h, f * FH:(f + 1) * FH],
                                         start=(h == 0), stop=(h == H - 1))
                    if (m + f) % 2 == 0:
                        nc.vector.tensor_copy(m_sb[:, m, f * FH:(f + 1) * FH],
                                              m_ps[:, :])
                    else:
                        nc.scalar.activation(m_sb[:, m, f * FH:(f + 1) * FH],
                                             m_ps[:, :],
                                             mybir.ActivationFunctionType.Copy)

            # ---- phase 4: y = x_half @ M (y stored packed, 2 tiles/store) ----
            for t in range(XTT):
                xt_t = xts.tile([128, KC, 512], BF16, tag="xt")
                nc.gpsimd.dma_start(out=xt_t[:, :, :], in_=xtq_d[:, t, :, :])
                for sp in range(2):
                    y_sb = yo.tile([128, 2, C], BF16, tag="y")
                    for j in range(2):
                        sub = sp * 2 + j
                        for f in range(2):
                            y_ps = ps.tile([128, FH], F32, tag="ps")
                            for k in range(KC):
                                nc.tensor.matmul(
                                    y_ps[:, :],
                                    xt_t[:, k, 128 * sub:128 * (sub + 1)],
                                    m_sb[:, k, f * FH:(f + 1) * FH],
                                    start=(k == 0), stop=(k == KC - 1))
                            if f == 0:
                                nc.vector.tensor_copy(
                                    y_sb[:, j, f * FH:(f + 1) * FH], y_ps[:, :])
                            else:
                                nc.scalar.activation(
                                    y_sb[:, j, f * FH:(f + 1) * FH], y_ps[:, :],
                                    mybir.ActivationFunctionType.Copy)
                    yeng = nc.sync if sp == 0 else nc.scalar
                    yeng.dma_start(out=y_d[:, t * 2 + sp, :, :],
                                   in_=y_sb[:, :, :])

    nc.compile()
    return nc


def _get_program(has_bias: bool = False):
    if "v3" not in _CACHE:
        _CACHE["v3"] = _build()
    return _CACHE["v3"]


def _prepare_inputs(x, Wqkv, bqkv, temperature, Wproj, bproj, has_bias=False):
    """Build the 8 per-core input maps (host-side packing + dtype prep)."""
    x = np.asarray(x, np.float32)
    Wqkv = np.asarray(Wqkv, np.float32)
    temperature = np.asarray(temperature, np.float32)
    Wproj = np.asarray(Wproj, np.float32)

    Wq = Wqkv[:, :C]
    Wk = Wqkv[:, C:2 * C]
    Wv = Wqkv[:, 2 * C:]
    # [128, KC, C]: wqp[p, k, c] = Wq[k*128+p, c]
    wqp = np.ascontiguousarray(
        Wq.reshape(KC, 128, C).transpose(1, 0, 2)).astype(BFNP)
    wkp = np.ascontiguousarray(
        Wk.reshape(KC, 128, C).transpose(1, 0, 2)).astype(BFNP)
    # wvth[e, h, c] = Wv[c, 64h+e];  wprojh[d, h, c] = Wproj[64h+d, c]
    wvth = np.ascontiguousarray(
        Wv.reshape(C, H, HD).transpose(2, 1, 0)).astype(BFNP)
    wprojh = np.ascontiguousarray(
        Wproj.reshape(H, HD, C).transpose(1, 0, 2)).astype(BFNP)
    tempdh = np.tile(temperature.reshape(1, H), (HD, 1)).astype(np.float32)

    common = dict(wq=wqp, wk=wkp, wvth=wvth, wprojh=wprojh, tempdh=tempdh,
                  ones128=np.ones((128, 1), BFNP),
                  ones1=np.ones((1, HD), BFNP),
                  ident=np.eye(128, dtype=np.float32).astype(BFNP))

    # xq[p, g, j, c] = x[b, g*512 + j*128 + p, c] — shared by the batch pair
    xqs = [np.ascontiguousarray(
        x[b].reshape(NG, 4, 128, C).transpose(2, 0, 1, 3)).astype(BFNP)
        for b in range(B)]

    in_maps = []
    for core in range(NCORES):
        b, j = core // 2, core % 2
        xh = x[b, j * NLOC:(j + 1) * NLOC]
        # xtq[p, t, k, n] = xh[t*512 + n, k*128 + p]
        xtq = np.ascontiguousarray(
            xh.reshape(XTT, 512, KC, 128).transpose(3, 0, 2, 1)).astype(BFNP)
        m = dict(common)
        m["xq"] = xqs[b]
        m["xtq"] = xtq
        in_maps.append(m)
    return in_maps


def _reference_host(x, Wqkv, bqkv, temperature, Wproj, bproj):
    """Exact numpy fallback (used only for nonzero biases)."""
    x = np.asarray(x, np.float64)
    Wqkv = np.asarray(Wqkv, np.float64)
    bqkv = np.asarray(bqkv, np.float64)
    temperature = np.asarray(temperature, np.float64)
    Wproj = np.asarray(Wproj, np.float64)
    bproj = np.asarray(bproj, np.float64)
    Bq, Nq, Cq = x.shape
    hd = Cq // H
    qkv = (x @ Wqkv + bqkv).reshape(Bq, Nq, 3, H, hd)
    qkv = qkv.transpose(2, 0, 3, 4, 1)
    q, k, v = qkv[0], qkv[1], qkv[2]

    def l2n(t):
        n = np.sqrt((t * t).sum(axis=-1, keepdims=True))
        return t / np.maximum(n, 1e-12)

    q = l2n(q) * temperature
    k = l2n(k)
    logits = np.einsum('bhdn,bhen->bhde', q, k)
    logits -= logits.max(axis=-1, keepdims=True)
    e = np.exp(logits)
    attn = e / e.sum(axis=-1, keepdims=True)
    out = np.einsum('bhde,bhen->bhdn', attn, v)
    out = out.transpose(0, 3, 1, 2).reshape(Bq, Nq, Cq)
    return (out @ Wproj + bproj).astype(np.float32)


def kernel(x, Wqkv, bqkv, temperature, Wproj, bproj):
    has_bias = bool(np.any(np.asarray(bqkv)) or np.any(np.asarray(bproj)))
    if has_bias:
        return _reference_host(x, Wqkv, bqkv, temperature, Wproj, bproj)
    from concourse import bass2jax
    nc = _get_program(False)
    in_maps = _prepare_inputs(x, Wqkv, bqkv, temperature, Wproj, bproj, False)
    results = bass2jax.run_bass_via_pjrt(nc, in_maps, n_cores=NCORES)
    out = np.empty((B, N, C), np.float32)
    for core in range(NCORES):
        b, j = core // 2, core % 2
        # y packed [128, 16, 2, C]: row = s*256 + jj*128 + p
        yp = results[core]["y"].astype(np.float32)
        out[b, j * NLOC:(j + 1) * NLOC, :] = (
            yp.transpose(1, 2, 0, 3).reshape(NLOC, C))
    return out
